# revision 1
# baseline (speedup 1.0000x reference)
"""DiffusionTransformer3D-CLS Trainium2 kernel.

Data-parallel over B=8 across 8 NeuronCores (one batch element per core,
weights replicated). Per core the token state lives in SBUF transposed as
two [128d, 5160L] f32r tiles; all matmuls run in float32r (fp22) at full
PE rate. LayerNorm stats are computed with ones-matmuls on the PE and
per-token scalars are partition-broadcast on GpSimd. The cls cross
attention uses a block-diagonal Q matrix to produce all 8 head scores in
one matmul column, a single fused exp+accumulate softmax pass, and
PE-transposed probability chunks against streamed V chunks. The token MLP
streams 512-wide L chunks: W1' (modulation folded into the weights),
gelu with fused bias on ScalarE, W2, and a fused residual via
scalar_tensor_tensor; the mlp output bias is folded in as a K=1 rank-1
matmul into PSUM.
"""

import sys

for _p in ("/opt/trn_rl_repo", "/opt/pypackages"):
    if _p not in sys.path:
        sys.path.append(_p)

import numpy as np
import concourse.bass as bass
import concourse.tile as tile
from concourse import mybir
from concourse.bass_utils import run_bass_kernel_spmd

AF = mybir.ActivationFunctionType
ALU = mybir.AluOpType
F32R = mybir.dt.float32r
F32 = mybir.dt.float32
I32 = mybir.dt.int32

B, L, C, D, H, DEPTH, FF, TE = 8, 5160, 2, 256, 8, 8, 1024, 256
HD = D // H  # 32
NCORES = 8
EPS = 1e-5
PI = float(np.pi)

# 512-wide L chunks for the streamed phases (stats, K/scores, MLP)
CHUNKS = [(i * 512, 512) for i in range(10)] + [(5120, 40)]
# 128-wide L chunks for V / probability transposes / attention apply
LCH = [(i * 128, 128) for i in range(40)] + [(5120, 40)]
NL = len(LCH)

TRACE = False
LAST = None


def split_excess_waits(nc, limit=1):
    """Walrus rejects instructions carrying more sem waits than the ISA
    struct holds (the fused-LW f32r matmul takes just one). Hoist excess
    on_wait entries into preceding same-engine InstNoOps; same-queue
    program order preserves semantics."""
    fn = nc.m.functions[0]
    blocks = getattr(fn, "instruction_blocks", None) or getattr(fn, "blocks")
    for bb in blocks:
        insts = bb.instructions
        out = []
        for inst in insts:
            si = inst.sync_info
            waits = list(si.on_wait) if si is not None and si.on_wait else []
            if len(waits) > limit:
                keep = waits[-limit:]
                excess = waits[:-limit]
                for i in range(0, len(excess), limit):
                    nop = mybir.InstNoOp(
                        name=nc.get_next_instruction_name(),
                        sync_info=mybir.SyncInfo(
                            on_wait=excess[i:i + limit], on_update=[]
                        ),
                        bass_nofuse=True,
                        engine=inst.engine,
                    )
                    nc.register_instruction(nop)
                    out.append(nop)
                si.on_wait = keep
            out.append(inst)
        if len(out) != len(insts):
            insts[:] = out
    return nc


def build_nc(depth=DEPTH, debug=False):
    nc = bass.Bass(target_bir_lowering=False, trn_type="TRN2")
    V = nc.vector
    S = nc.scalar
    G = nc.gpsimd
    T = nc.tensor

    def mmv(out, lhsT, rhs, start, stop):
        # N=1 matvecs: walrus rejects f32r matmuls with a single moving
        # column; run these tiny ops as plain fp32 (4-pass, full precision).
        T.matmul(out, lhsT.bitcast(F32), rhs.bitcast(F32),
                 start=start, stop=stop)

    # ---- DRAM tensors (per-core inputs first, then replicated weights) ----
    d_xT = nc.dram_tensor("xT", [C, L], F32R, kind="ExternalInput")
    d_t = nc.dram_tensor("tval", [1, 1], I32, kind="ExternalInput")
    d_posT = nc.dram_tensor("posT", [D, L], F32, kind="ExternalInput")
    d_inwT = nc.dram_tensor("inwT", [C, D], F32R, kind="ExternalInput")
    d_inb = nc.dram_tensor("inb", [D, 1], F32, kind="ExternalInput")
    d_freqs = nc.dram_tensor("freqs", [TE // 2, 2], F32, kind="ExternalInput")
    d_tp1T = nc.dram_tensor("tp1T", [TE, D], F32R, kind="ExternalInput")
    d_tp1b = nc.dram_tensor("tp1b", [D, 1], F32, kind="ExternalInput")
    d_tp2T = nc.dram_tensor("tp2T", [D, D], F32R, kind="ExternalInput")
    d_tp2b = nc.dram_tensor("tp2b", [D, 1], F32, kind="ExternalInput")
    d_cls = nc.dram_tensor("clsv", [D, 1], F32R, kind="ExternalInput")
    d_qkvoT = nc.dram_tensor("qkvoT", [DEPTH, 4, D, D], F32R, kind="ExternalInput")
    d_attnb = nc.dram_tensor("attnb", [DEPTH, 4, D, 1], F32, kind="ExternalInput")
    d_modT = nc.dram_tensor("modT", [DEPTH, 3, D, 3 * D], F32R, kind="ExternalInput")
    d_modb = nc.dram_tensor("modb", [DEPTH, 3, 3 * D, 1], F32, kind="ExternalInput")
    d_lng = nc.dram_tensor("lng", [DEPTH, 3, D, 1], F32, kind="ExternalInput")
    d_lnb = nc.dram_tensor("lnb", [DEPTH, 3, D, 1], F32, kind="ExternalInput")
    d_w1T = nc.dram_tensor("w1T", [DEPTH, 2, D, FF], F32R, kind="ExternalInput")
    d_b1 = nc.dram_tensor("b1", [DEPTH, 2, FF, 1], F32, kind="ExternalInput")
    d_w2T = nc.dram_tensor("w2T", [DEPTH, 2, FF, D], F32R, kind="ExternalInput")
    d_b2 = nc.dram_tensor("b2", [DEPTH, 2, D, 1], F32, kind="ExternalInput")
    d_fing = nc.dram_tensor("fing", [D, 1], F32, kind="ExternalInput")
    d_finb = nc.dram_tensor("finb", [D, 1], F32, kind="ExternalInput")
    d_outwT = nc.dram_tensor("outwT", [D, C], F32R, kind="ExternalInput")
    d_outb = nc.dram_tensor("outb", [C, 1], F32, kind="ExternalInput")
    d_ident = nc.dram_tensor("ident", [8, 8], F32R, kind="ExternalInput")
    d_onessc = nc.dram_tensor("onessc", [128, 1], F32R, kind="ExternalInput")  # 1/256
    d_onesrow = nc.dram_tensor("onesrow", [1, 512], F32R, kind="ExternalInput")
    d_qzero = nc.dram_tensor("qzero", [128, 8], F32R, kind="ExternalInput")
    d_outT = nc.dram_tensor("outT", [C, L], F32, kind="ExternalOutput")
    if debug:
        d_dbg_temb = nc.dram_tensor("dbg_temb", [D, 1], F32, kind="ExternalOutput")
        d_dbg_emb = nc.dram_tensor("dbg_emb", [D, L], F32, kind="ExternalOutput")
        d_dbg_mu = nc.dram_tensor("dbg_mu", [1, L], F32, kind="ExternalOutput")
        d_dbg_r = nc.dram_tensor("dbg_r", [1, L], F32, kind="ExternalOutput")
        d_dbg_p = nc.dram_tensor("dbg_p", [8, L], F32, kind="ExternalOutput")
        d_dbg_asb = nc.dram_tensor("dbg_asb", [8, D], F32, kind="ExternalOutput")
        d_dbg_cls = nc.dram_tensor("dbg_cls", [D, 1], F32, kind="ExternalOutput")
        d_dbg_mv = nc.dram_tensor("dbg_mv", [128, 6], F32, kind="ExternalOutput")
        d_dbg_tok = nc.dram_tensor("dbg_tok", [D, L], F32, kind="ExternalOutput")
        d_dbg_qm = nc.dram_tensor("dbg_qm", [D, 8], F32, kind="ExternalOutput")
        d_dbg_sin = nc.dram_tensor("dbg_sin", [D, 1], F32, kind="ExternalOutput")
        d_dbg_m = nc.dram_tensor("dbg_m", [D, 1], F32, kind="ExternalOutput")
        d_dbg_sraw = nc.dram_tensor("dbg_sraw", [8, L], F32, kind="ExternalOutput")
        d_dbg_ang = nc.dram_tensor("dbg_ang", [128, 4], F32, kind="ExternalOutput")
        d_dbg_f1 = nc.dram_tensor("dbg_f1", [128, 1], F32, kind="ExternalOutput")
        d_dbg_f2 = nc.dram_tensor("dbg_f2", [128, 1], F32, kind="ExternalOutput")
        d_dbg_hc = nc.dram_tensor("dbg_hc", [D, 1], F32, kind="ExternalOutput")
        d_dbg_ab = nc.dram_tensor("dbg_ab", [128, 4], F32, kind="ExternalOutput")
        d_dbg_mc = nc.dram_tensor("dbg_mc", [1, 4], F32, kind="ExternalOutput")
        d_dbg_qp = nc.dram_tensor("dbg_qp", [D, 1], F32, kind="ExternalOutput")

    def col2(dram_ap, groups):
        """[(g p), 1] dram vector -> [128, g] AP for DMA."""
        return dram_ap[:, 0].rearrange("(g p) -> p g", p=128)

    def ld_split(dst, dram2d, g):
        """dram (g*128, X) -> tile [128, g*X] with block g at cols g*X."""
        x = dram2d.shape[1]
        nc.sync.dma_start(
            dst.rearrange("p (g x) -> p g x", g=g),
            dram2d.rearrange("(g p) x -> p g x", p=128))

    with tile.TileContext(nc) as tc:
        with tc.tile_pool(name="state", bufs=1) as st, \
             tc.tile_pool(name="wts", bufs=1) as wp, \
             tc.tile_pool(name="wts1", bufs=1) as wp1, \
             tc.tile_pool(name="vecs", bufs=2) as vp, \
             tc.tile_pool(name="chk", bufs=2) as cp, \
             tc.tile_pool(name="achk", bufs=1) as apool, \
             tc.tile_pool(name="ph1", bufs=2, space="PSUM") as ph1, \
             tc.tile_pool(name="ph2", bufs=2, space="PSUM") as ph2, \
             tc.tile_pool(name="pmisc", bufs=4, space="PSUM") as pm:

            # ---------------- persistent state tiles ----------------
            tokT = [st.tile([128, L], F32R, tag=f"tok{d}", name=f"tok{d}") for d in range(2)]
            p_sc = st.tile([8, L], F32R, tag="scores", name="scores")       # scores then probs
            mu_r = st.tile([1, L], F32R, tag="mu_r", name="mu_r")
            r_r = st.tile([1, L], F32R, tag="r_r", name="r_r")
            pT = st.tile([128, NL * 8], F32R, tag="pT", name="pT")
            cls = [st.tile([128, 1], F32R, tag=f"cls{d}", name=f"cls{d}") for d in range(2)]
            temb = [st.tile([128, 1], F32R, tag=f"temb{d}", name=f"temb{d}") for d in range(2)]
            stm = [st.tile([128, 1], F32R, tag=f"stm{d}", name=f"stm{d}") for d in range(2)]
            # precomputed mod vectors for adaln groups 0/1 (temb only):
            # col = i*12 + g*6 + v*2 + dt, v in (s, sh, gate)
            modpre = st.tile([128, DEPTH * 12], F32, tag="modpre", name="modpre")
            # precomputed (1+s)*gamma / (1+s)*beta+sh for groups 0/1:
            # col = i*8 + g*4 + a_or_b*2 + dt
            abpre = st.tile([128, DEPTH * 8], F32, tag="abpre", name="abpre")
            identsb = st.tile([8, 8], F32R, tag="ident", name="ident")
            onessc = st.tile([128, 1], F32R, tag="onessc", name="onessc")
            onesrow = st.tile([1, 512], F32R, tag="onesrow", name="onesrow")
            lngsb = st.tile([128, DEPTH * 6], F32, tag="lngsb", name="lngsb")  # i*6+g*2+dt
            lnbsb = st.tile([128, DEPTH * 6], F32, tag="lnbsb", name="lnbsb")

            nc.sync.dma_start(identsb[:], d_ident[:, :])
            nc.sync.dma_start(onessc[:], d_onessc[:, :])
            nc.sync.dma_start(onesrow[:], d_onesrow[:, :])
            for dt in range(2):
                nc.sync.dma_start(
                    lngsb[:].rearrange("p (i g dt) -> p i g dt",
                                       i=DEPTH, g=3)[:, :, :, dt],
                    d_lng[:, :, 128 * dt:128 * dt + 128, 0].rearrange(
                        "i g p -> p i g"))
                nc.sync.dma_start(
                    lnbsb[:].rearrange("p (i g dt) -> p i g dt",
                                       i=DEPTH, g=3)[:, :, :, dt],
                    d_lnb[:, :, 128 * dt:128 * dt + 128, 0].rearrange(
                        "i g p -> p i g"))
            nc.sync.dma_start(cls[0][:], d_cls[0:128, :])
            nc.sync.dma_start(cls[1][:], d_cls[128:256, :])

            # ---------------- time embedding ----------------
            with tc.tile_pool(name="pre", bufs=1) as pre:
                tfl = pre.tile([1, 1], F32R, tag="tfl", name="tfl")
                traw = pre.tile([1, 1], I32, tag="traw", name="traw")
                nc.sync.dma_start(traw[:], d_t[:, :])
                V.tensor_copy(tfl[:], traw[:])
                tb = pm.tile([128, 1], F32, tag="ps", name="tb")
                mmv(tb[:], onesrow[0:1, 0:128], tfl[:], start=True, stop=True)
                # freqs arrive split hi/lo: hi is exactly representable in
                # fp22 (immune to the input-upload rounding observed on this
                # path); ang = t*hi + t*lo restores full fp32 precision.
                fsb = pre.tile([128, 2], F32, tag="fsb", name="fsb")
                nc.sync.dma_start(fsb[:], d_freqs[:, :])
                if debug:
                    nc.sync.dma_start(d_dbg_f1[:, :], fsb[:, 0:1])
                    nc.sync.dma_start(d_dbg_f2[:, :], fsb[:, 1:2])
                ang = pre.tile([128, 1], F32, tag="ang", name="ang")
                ang2 = pre.tile([128, 1], F32, tag="ang2", name="ang2")
                V.tensor_tensor(ang[:], tb[:], fsb[:, 0:1], ALU.mult)
                V.tensor_tensor(ang2[:], tb[:], fsb[:, 1:2], ALU.mult)
                V.tensor_tensor(ang[:], ang[:], ang2[:], ALU.add)
                dsc = pre.tile([128, 1], F32, tag="dsc", name="dsc")
                qi = pre.tile([128, 1], I32, tag="qi", name="qi")
                qf = pre.tile([128, 1], F32, tag="qf", name="qf")
                msk = pre.tile([128, 1], F32, tag="msk", name="msk")
                TWO_PI = 2 * PI

                def mod2pi(dst, shift):
                    # dst = mod(ang + shift, 2*pi) - pi, robust to whether the
                    # f32->i32 convert rounds or truncates (the +-2pi fixups
                    # absorb a +-1 error in the quotient estimate).
                    V.tensor_scalar(dst[:], ang[:], shift, None, ALU.add)
                    V.tensor_scalar(dsc[:], dst[:], 1.0 / TWO_PI, 0.5,
                                    ALU.mult, ALU.subtract)
                    V.tensor_copy(qi[:], dsc[:])
                    V.tensor_copy(qf[:], qi[:])
                    V.scalar_tensor_tensor(dst[:], qf[:], -TWO_PI, dst[:],
                                           ALU.mult, ALU.add)
                    V.tensor_scalar(msk[:], dst[:], TWO_PI, None, ALU.is_ge)
                    V.scalar_tensor_tensor(dst[:], msk[:], -TWO_PI, dst[:],
                                           ALU.mult, ALU.add)
                    V.tensor_scalar(msk[:], dst[:], 0.0, None, ALU.is_lt)
                    V.scalar_tensor_tensor(dst[:], msk[:], TWO_PI, dst[:],
                                           ALU.mult, ALU.add)
                    V.tensor_scalar(dst[:], dst[:], PI, None, ALU.subtract)

                m1 = pre.tile([128, 1], F32, tag="m1", name="m1")
                mod2pi(m1, PI)
                m2 = pre.tile([128, 1], F32, tag="m2", name="m2")
                mod2pi(m2, 1.5 * PI)
                sinf = pre.tile([128, 1], F32R, tag="sinf", name="sinf")
                cosf = pre.tile([128, 1], F32R, tag="cosf", name="cosf")
                S.activation(sinf[:], m1[:], AF.Sin)
                S.activation(cosf[:], m2[:], AF.Sin)
                if debug:
                    nc.sync.dma_start(d_dbg_ang[:, 0:1], ang[:])
                    nc.sync.dma_start(d_dbg_ang[:, 1:2], dsc[:])
                    nc.sync.dma_start(d_dbg_ang[:, 2:3], qf[:])
                    nc.sync.dma_start(d_dbg_ang[:, 3:4], fsb[:, 0:1])
                    nc.sync.dma_start(d_dbg_sin[0:128, :].bitcast(F32R), sinf[:])
                    nc.sync.dma_start(d_dbg_sin[128:256, :].bitcast(F32R), cosf[:])
                    nc.sync.dma_start(d_dbg_m[0:128, :], m1[:])
                    nc.sync.dma_start(d_dbg_m[128:256, :], m2[:])

                ttp1 = pre.tile([128, 512], F32R, tag="ttp1", name="ttp1")
                ld_split(ttp1[:], d_tp1T, 2)
                ttp2 = pre.tile([128, 512], F32R, tag="ttp2", name="ttp2")
                ld_split(ttp2[:], d_tp2T, 2)
                tp1b = pre.tile([128, 2], F32, tag="tp1b", name="tp1b")
                nc.sync.dma_start(tp1b[:], col2(d_tp1b, 2))
                tp2b = pre.tile([128, 2], F32, tag="tp2b", name="tp2b")
                nc.sync.dma_start(tp2b[:], col2(d_tp2b, 2))

                st1 = [pre.tile([128, 1], F32R, tag=f"st1{m}", name=f"st1{m}") for m in range(2)]
                for m in range(2):
                    ps = pm.tile([128, 1], F32, tag="ps", name="ps")
                    mmv(ps[:], ttp1[:, 128 * m:128 * m + 128], sinf[:],
                             start=True, stop=False)
                    mmv(ps[:], ttp1[:, 256 + 128 * m:256 + 128 * m + 128],
                             cosf[:], start=False, stop=True)
                    S.activation(st1[m][:], ps[:], AF.Silu, bias=tp1b[:, m:m + 1])
                for m in range(2):
                    ps = pm.tile([128, 1], F32, tag="ps", name="ps")
                    mmv(ps[:], ttp2[:, 128 * m:128 * m + 128], st1[0][:],
                             start=True, stop=False)
                    mmv(ps[:], ttp2[:, 256 + 128 * m:256 + 128 * m + 128],
                             st1[1][:], start=False, stop=True)
                    S.activation(temb[m][:], ps[:], AF.Identity,
                                 bias=tp2b[:, m:m + 1])
                    S.activation(stm[m][:], temb[m][:], AF.Silu)

                # adaln groups 0/1 mod vectors for every layer (temb only)
                for i in range(depth):
                    for g in range(2):
                        tmg = pre.tile([128, 1536], F32R, tag="tmg", name="tmg")
                        ld_split(tmg[:], d_modT[i, g], 2)
                        tmb = pre.tile([128, 6], F32, tag="tmb", name="tmb")
                        nc.sync.dma_start(tmb[:], col2(d_modb[i, g], 6))
                        for m in range(6):
                            ps = pm.tile([128, 1], F32, tag="ps", name="ps")
                            mmv(ps[:], tmg[:, 128 * m:128 * m + 128],
                                     stm[0][:], start=True, stop=False)
                            mmv(ps[:], tmg[:, 768 + 128 * m:768 + 128 * m + 128],
                                     stm[1][:], start=False, stop=True)
                            colm = i * 12 + g * 6 + m
                            V.tensor_scalar(modpre[:, colm:colm + 1], ps[:],
                                            tmb[:, m:m + 1], None, ALU.add)
                        # (1+s)*gamma, (1+s)*beta + sh
                        scr = pre.tile([128, 1], F32, tag="scr", name="scr")
                        for dt in range(2):
                            scol = i * 12 + g * 6 + dt
                            shcol = i * 12 + g * 6 + 2 + dt
                            lcol = i * 6 + g * 2 + dt
                            acol = i * 8 + g * 4 + dt
                            bcol = i * 8 + g * 4 + 2 + dt
                            V.tensor_scalar(scr[:], modpre[:, scol:scol + 1],
                                            1.0, None, ALU.add)
                            V.tensor_tensor(abpre[:, acol:acol + 1], scr[:],
                                            lngsb[:, lcol:lcol + 1], ALU.mult)
                            V.tensor_tensor(abpre[:, bcol:bcol + 1], scr[:],
                                            lnbsb[:, lcol:lcol + 1], ALU.mult)
                            V.tensor_tensor(abpre[:, bcol:bcol + 1],
                                            abpre[:, bcol:bcol + 1],
                                            modpre[:, shcol:shcol + 1], ALU.add)

            # ---------------- embed: tokens = x @ in_w.T + in_b + pos ----
            with tc.tile_pool(name="pre2", bufs=1) as pre:
                inwsb = pre.tile([C, D], F32R, tag="inwsb", name="inwsb")
                nc.sync.dma_start(inwsb[:], d_inwT[:, :])
                inbsb = pre.tile([128, 2], F32, tag="inbsb", name="inbsb")
                nc.sync.dma_start(inbsb[:], col2(d_inb, 2))
                for (o, w) in CHUNKS:
                    xtc = pre.tile([C, 512], F32R, tag="xtc", name="xtc")
                    nc.sync.dma_start(xtc[:, 0:w], d_xT[:, o:o + w])
                    for dt in range(2):
                        ppc = pre.tile([128, 512], F32, tag=f"ppc{dt}", name=f"ppc{dt}")
                        nc.sync.dma_start(ppc[:, 0:w],
                                          d_posT[128 * dt:128 * dt + 128, o:o + w])
                        ps = pm.tile([128, 512], F32, tag="ps", name="ps")
                        T.matmul(ps[:, 0:w], inwsb[:, 128 * dt:128 * dt + 128],
                                 xtc[:, 0:w], start=True, stop=True)
                        V.scalar_tensor_tensor(
                            tokT[dt][:, o:o + w], ps[:, 0:w],
                            inbsb[:, dt:dt + 1], ppc[:, 0:w], ALU.add, ALU.add)

            if debug:
                for dt in range(2):
                    nc.sync.dma_start(
                        d_dbg_emb[128 * dt:128 * dt + 128, :].bitcast(F32R),
                        tokT[dt][:, :])
                nc.sync.dma_start(d_dbg_temb[0:128, :].bitcast(F32R), temb[0][:])
                nc.sync.dma_start(d_dbg_temb[128:256, :].bitcast(F32R), temb[1][:])

            # ---------------- per-layer tiles & emission ----------------
            def ln_stats():
                """Populate stats rows (0=mu, 2=r) from current tokT."""
                for (o, w) in CHUNKS:
                    ps_mu = pm.tile([1, 512], F32, tag="ps", name="ps_mu")
                    ps_m2 = pm.tile([1, 512], F32, tag="ps", name="ps_m2")
                    for dt in range(2):
                        tsq = cp.tile([128, 512], F32R, tag=f"big{dt}", name=f"tsq{dt}")
                        S.activation(tsq[:, 0:w], tokT[dt][:, o:o + w], AF.Square)
                        T.matmul(ps_mu[:, 0:w], onessc[:], tokT[dt][:, o:o + w],
                                 start=(dt == 0), stop=(dt == 1))
                        T.matmul(ps_m2[:, 0:w], onessc[:], tsq[:, 0:w],
                                 start=(dt == 0), stop=(dt == 1))
                    V.tensor_copy(mu_r[0:1, o:o + w], ps_mu[:, 0:w])
                    # per-chunk: musq -> sq ; var = (m2+eps) - musq -> var ;
                    # 1/var ; r = sqrt(1/var) -> r_r
                    sq = cp.tile([1, 512], F32, tag="mis0", name="sq")
                    S.activation(sq[0:1, 0:w], mu_r[0:1, o:o + w], AF.Square)
                    var = cp.tile([1, 512], F32, tag="mis1", name="var")
                    V.scalar_tensor_tensor(var[0:1, 0:w], ps_m2[:, 0:w], EPS,
                                           sq[0:1, 0:w], ALU.add, ALU.subtract)
                    V.reciprocal(var[0:1, 0:w], var[0:1, 0:w])
                    S.activation(r_r[0:1, o:o + w], var[0:1, 0:w], AF.Sqrt)

            dbg_mc_done = [False]

            def cls_ln(i, g, out_tag):
                """h = (cls - mu)*r*(1+s)*gamma + ((1+s)*beta + sh), returns
                two [128,1] f32r tiles."""
                csc = [vp.tile([128, 2], F32R, tag=f"csc{d}", name=f"csc{d}") for d in range(2)]
                for d in range(2):
                    V.tensor_copy(csc[d][:, 0:1], onessc[:])
                    V.tensor_scalar(csc[d][:, 1:2], cls[d][:], 1.0 / 256, None,
                                    ALU.mult)
                ps = pm.tile([1, 2], F32, tag="ps", name="ps")
                for d in range(2):
                    mmv(ps[0:1, 0:2], cls[d][:], csc[d][:, 0:2],
                        start=(d == 0), stop=(d == 1))
                mc = vp.tile([1, 4], F32R, tag="mc", name="mc")
                V.tensor_copy(mc[0:1, 0:2], ps[0:1, 0:2])
                S.activation(mc[0:1, 2:3], mc[0:1, 0:1], AF.Square)
                V.scalar_tensor_tensor(mc[0:1, 3:4], mc[0:1, 1:2], EPS,
                                       mc[0:1, 2:3], ALU.add, ALU.subtract)
                with nc.allow_low_precision(reason="fp22 cls LN stats"):
                    V.reciprocal(mc[0:1, 3:4], mc[0:1, 3:4])
                S.activation(mc[0:1, 3:4], mc[0:1, 3:4], AF.Sqrt)
                if debug and not dbg_mc_done[0]:
                    dbg_mc_done[0] = True
                    nc.sync.dma_start(d_dbg_mc[:, :].bitcast(F32R), mc[0:1, :])
                mcb = pm.tile([128, 1], F32, tag="ps", name="mcb")
                rcb = pm.tile([128, 1], F32, tag="ps", name="rcb")
                mmv(mcb[:], onesrow[0:1, 0:128], mc[0:1, 0:1],
                         start=True, stop=True)
                mmv(rcb[:], onesrow[0:1, 0:128], mc[0:1, 3:4],
                         start=True, stop=True)
                hc = [vp.tile([128, 1], F32R, tag=f"{out_tag}{d}", name=f"{out_tag}{d}") for d in range(2)]
                for d in range(2):
                    acol = i * 8 + g * 4 + d
                    bcol = i * 8 + g * 4 + 2 + d
                    V.tensor_tensor(hc[d][:], cls[d][:], mcb[:], ALU.subtract)
                    V.tensor_tensor(hc[d][:], hc[d][:], rcb[:], ALU.mult)
                    V.scalar_tensor_tensor(hc[d][:], hc[d][:],
                                           abpre[:, acol:acol + 1],
                                           abpre[:, bcol:bcol + 1],
                                           ALU.mult, ALU.add)
                return hc

            for i in range(depth):
                # ---- weight loads ----
                qkvo = wp1.tile([128, 2048], F32R, tag="qkvo", name="qkvo")
                for dt in range(2):
                    nc.sync.dma_start(
                        qkvo[:, 1024 * dt:1024 * dt + 1024].rearrange(
                            "p (w x) -> p w x", w=4),
                        d_qkvoT[i][:, 128 * dt:128 * dt + 128, :].rearrange(
                            "w p x -> p w x"))
                w1 = wp.tile([128, 2048], F32R, tag="w1", name="w1")
                ld_split(w1[:], d_w1T[i, 1], 2)
                w2 = wp.tile([128, 2048], F32R, tag="w2", name="w2")
                ld_split(w2[:], d_w2T[i, 1], 8)
                w1c = wp1.tile([128, 2048], F32R, tag="w1c", name="w1c")
                ld_split(w1c[:], d_w1T[i, 0], 2)
                w2c = wp1.tile([128, 2048], F32R, tag="w2c", name="w2c")
                ld_split(w2c[:], d_w2T[i, 0], 8)
                mod2 = wp1.tile([128, 1536], F32R, tag="mod2", name="mod2")
                ld_split(mod2[:], d_modT[i, 2], 2)
                tattnb = vp.tile([128, 8], F32, tag="tattnb", name="tattnb")  # col = w*2+dt
                nc.sync.dma_start(
                    tattnb[:].rearrange("p (w dt) -> p w dt", w=4),
                    d_attnb[i][:, :, 0].rearrange("w (dt p) -> p w dt", p=128))
                tb1 = vp.tile([128, 8], F32, tag="tb1", name="tb1")
                nc.sync.dma_start(tb1[:], col2(d_b1[i, 1], 8))
                tb1c = vp.tile([128, 8], F32, tag="tb1c", name="tb1c")
                nc.sync.dma_start(tb1c[:], col2(d_b1[i, 0], 8))
                b2row = vp.tile([1, 256], F32R, tag="b2row", name="b2row", bufs=1)
                nc.sync.dma_start(b2row[:], d_b2[i, 1].rearrange("d o -> o d").bitcast(F32R))
                tb2c = vp.tile([128, 2], F32, tag="tb2c", name="tb2c")
                nc.sync.dma_start(tb2c[:], col2(d_b2[i, 0], 2))
                tmodb2 = vp.tile([128, 6], F32, tag="tmodb2", name="tmodb2")
                nc.sync.dma_start(tmodb2[:], col2(d_modb[i, 2], 6))

                # ---- LN stats on block-input tokens (parallel with attn) ----
                ln_stats()

                if debug and i == 0:
                    nc.sync.dma_start(d_dbg_mu[:, :].bitcast(F32R), mu_r[0:1, :])
                    nc.sync.dma_start(d_dbg_r[:, :].bitcast(F32R), r_r[0:1, :])

                # ---- cls-attn: q from modulated LN(cls) ----
                hc = cls_ln(i, 0, "hca")
                if debug and i == 0:
                    nc.sync.dma_start(d_dbg_hc[0:128, :].bitcast(F32R), hc[0][:])
                    nc.sync.dma_start(d_dbg_hc[128:256, :].bitcast(F32R), hc[1][:])
                    for dd in range(2):
                        acol = 0 * 8 + 0 * 4 + dd
                        bcol = 0 * 8 + 0 * 4 + 2 + dd
                        nc.sync.dma_start(d_dbg_ab[:, dd:dd + 1],
                                          abpre[:, acol:acol + 1])
                        nc.sync.dma_start(d_dbg_ab[:, 2 + dd:3 + dd],
                                          abpre[:, bcol:bcol + 1])
                Qm = [vp.tile([128, 8], F32R, tag=f"qm{d}", name=f"qm{d}") for d in range(2)]
                for d in range(2):
                    qp = pm.tile([128, 1], F32, tag="ps", name="ps")
                    mmv(qp[:], qkvo[:, 1024 * 0 + 0 + 128 * d:1024 * 0 + 128 * d + 128],
                             hc[0][:], start=True, stop=False)
                    mmv(qp[:], qkvo[:, 1024 * 1 + 0 + 128 * d:1024 * 1 + 128 * d + 128],
                             hc[1][:], start=False, stop=True)
                    if debug and i == 0:
                        qps = vp.tile([128, 1], F32, tag="qps", name="qps")
                        V.tensor_copy(qps[:], qp[:])
                        nc.sync.dma_start(d_dbg_qp[128 * d:128 * d + 128, :],
                                          qps[:])
                    nc.sync.dma_start(Qm[d][:], d_qzero[:, :])
                    for hh in range(4):
                        r0 = 32 * hh
                        col = 4 * d + hh
                        V.tensor_scalar(Qm[d][r0:r0 + 32, col:col + 1],
                                        qp[r0:r0 + 32, 0:1],
                                        tattnb[r0:r0 + 32, 0 + d:d + 1],
                                        None, ALU.add)

                # ---- K chunks + scores ----
                for (o, w) in CHUNKS:
                    ksb = []
                    for m in range(2):
                        kp = pm.tile([128, 512], F32, tag="ps", name="ps")
                        for dt in range(2):
                            T.matmul(kp[:, 0:w],
                                     qkvo[:, 1024 * dt + 256 + 128 * m:
                                          1024 * dt + 256 + 128 * m + 128],
                                     tokT[dt][:, o:o + w],
                                     start=(dt == 0), stop=(dt == 1))
                        kt = cp.tile([128, 512], F32R, tag=f"mis{m}", name=f"k{m}")
                        V.tensor_copy(kt[:, 0:w], kp[:, 0:w])
                        ksb.append(kt)
                    sp = pm.tile([8, 512], F32, tag="ps", name="ps")
                    for m in range(2):
                        T.matmul(sp[:, 0:w], Qm[m][:], ksb[m][:, 0:w],
                                 start=(m == 0), stop=(m == 1))
                    V.tensor_copy(p_sc[:, o:o + w], sp[:, 0:w])

                if debug and i == 0:
                    nc.sync.dma_start(d_dbg_sraw[:, :].bitcast(F32R), p_sc[:, :])

                # ---- softmax (unnormalized; 1/sum folded in later) ----
                smax = vp.tile([8, 1], F32, tag="smax", name="smax")
                V.tensor_reduce(smax[:], p_sc[:, :], mybir.AxisListType.X, ALU.max)
                nb = vp.tile([8, 1], F32, tag="nb", name="nb")
                V.tensor_scalar(nb[:], smax[:], -1.0 / np.sqrt(HD), None, ALU.mult)
                ssum = vp.tile([8, 1], F32, tag="ssum", name="ssum")
                S.activation(p_sc[:, :], p_sc[:, :], AF.Exp,
                             bias=nb[:], scale=float(1.0 / np.sqrt(HD)),
                             accum_out=ssum[:])
                srec = vp.tile([8, 1], F32, tag="srec", name="srec")
                V.reciprocal(srec[:], ssum[:])

                # ---- transpose probs; stream V chunks; apply ----
                app = pm.tile([8, 256], F32, tag="ps", name="ps")
                for ci, (o, w) in enumerate(LCH):
                    tp = pm.tile([128, 8], F32R, tag="ps", name="ps")
                    T.transpose(tp[0:w, 0:8], p_sc[:, o:o + w], identsb[0:8, 0:8])
                    V.tensor_copy(pT[0:w, 8 * ci:8 * ci + 8], tp[0:w, 0:8])
                    vps = pm.tile([128, 256], F32, tag="ps", name="ps")
                    for dt in range(2):
                        T.matmul(vps[0:w, :], tokT[dt][:, o:o + w],
                                 qkvo[:, 1024 * dt + 512:1024 * dt + 768],
                                 start=(dt == 0), stop=(dt == 1))
                    vsb = cp.tile([128, 256], F32R, tag="vsb", name="vsb")
                    S.copy(vsb[0:w, :], vps[0:w, :])
                    T.matmul(app[:, :], pT[0:w, 8 * ci:8 * ci + 8], vsb[0:w, :],
                             start=(ci == 0), stop=(ci == NL - 1))

                asb = vp.tile([8, 256], F32R, tag="asb", name="asb")
                V.tensor_scalar(asb[:], app[:, :], srec[:], None, ALU.mult)
                if debug and i == 0:
                    nc.sync.dma_start(d_dbg_p[:, :].bitcast(F32R), p_sc[:, :])
                    nc.sync.dma_start(d_dbg_asb[:, :].bitcast(F32R), asb[:, :])
                    for d in range(2):
                        nc.sync.dma_start(
                            d_dbg_qm[128 * d:128 * d + 128, :].bitcast(F32R),
                            Qm[d][:, :])

                # ---- extract per-head blocks + bv; Wo matvec; cls update ----
                afl = [vp.tile([128, 1], F32R, tag=f"afl{d}", name=f"afl{d}") for d in range(2)]
                for d in range(2):
                    tpa = pm.tile([128, 8], F32R, tag="ps", name="ps")
                    T.transpose(tpa[0:128, 0:8], asb[:, 128 * d:128 * d + 128],
                                identsb[0:8, 0:8])
                    for hh in range(4):
                        r0 = 32 * hh
                        col = 4 * d + hh
                        V.tensor_scalar(afl[d][r0:r0 + 32, 0:1],
                                        tpa[r0:r0 + 32, col:col + 1],
                                        tattnb[r0:r0 + 32, 4 + d:4 + d + 1],
                                        None, ALU.add)
                for d in range(2):
                    op_ = pm.tile([128, 1], F32, tag="ps", name="ps")
                    mmv(op_[:], qkvo[:, 0 + 768 + 128 * d:768 + 128 * d + 128],
                             afl[0][:], start=True, stop=False)
                    mmv(op_[:], qkvo[:, 1024 + 768 + 128 * d:1024 + 768 + 128 * d + 128],
                             afl[1][:], start=False, stop=True)
                    gcol = i * 12 + 0 * 6 + 4 + d
                    V.scalar_tensor_tensor(cls[d][:], op_[:],
                                           modpre[:, gcol:gcol + 1],
                                           cls[d][:], ALU.mult, ALU.add)
                    bog = vp.tile([128, 1], F32, tag="bog", name="bog")
                    V.tensor_tensor(bog[:], tattnb[:, 6 + d:6 + d + 1],
                                    modpre[:, gcol:gcol + 1], ALU.mult)
                    V.tensor_tensor(cls[d][:], cls[d][:], bog[:], ALU.add)

                # ---- cls MLP ----
                hc2 = cls_ln(i, 1, "hcm")
                ac = [vp.tile([128, 1], F32R, tag=f"ac{m}", name=f"ac{m}") for m in range(8)]
                for m in range(8):
                    ps = pm.tile([128, 1], F32, tag="ps", name="ps")
                    mmv(ps[:], w1c[:, 128 * m:128 * m + 128], hc2[0][:],
                             start=True, stop=False)
                    mmv(ps[:], w1c[:, 1024 + 128 * m:1024 + 128 * m + 128],
                             hc2[1][:], start=False, stop=True)
                    S.activation(ac[m][:], ps[:], AF.Gelu, bias=tb1c[:, m:m + 1])
                for d in range(2):
                    ps = pm.tile([128, 1], F32, tag="ps", name="ps")
                    for k in range(8):
                        mmv(ps[:], w2c[:, 256 * k + 128 * d:256 * k + 128 * d + 128],
                                 ac[k][:], start=(k == 0), stop=(k == 7))
                    gcol = i * 12 + 1 * 6 + 4 + d
                    V.scalar_tensor_tensor(cls[d][:], ps[:],
                                           modpre[:, gcol:gcol + 1],
                                           cls[d][:], ALU.mult, ALU.add)
                    bog = vp.tile([128, 1], F32, tag="bog", name="bog")
                    V.tensor_tensor(bog[:], tb2c[:, d:d + 1],
                                    modpre[:, gcol:gcol + 1], ALU.mult)
                    V.tensor_tensor(cls[d][:], cls[d][:], bog[:], ALU.add)

                # ---- cond = temb + cls -> silu -> group-2 mod vectors ----
                sc2 = [vp.tile([128, 1], F32R, tag=f"sc2{d}", name=f"sc2{d}") for d in range(2)]
                for d in range(2):
                    cond = vp.tile([128, 1], F32R, tag=f"cond{d}", name=f"cond{d}")
                    V.tensor_tensor(cond[:], temb[d][:], cls[d][:], ALU.add)
                    S.activation(sc2[d][:], cond[:], AF.Silu)
                mvec = vp.tile([128, 6], F32, tag="mvec", name="mvec")  # s0,s1,sh0,sh1,g0,g1
                for m in range(6):
                    ps = pm.tile([128, 1], F32, tag="ps", name="ps")
                    mmv(ps[:], mod2[:, 128 * m:128 * m + 128], sc2[0][:],
                             start=True, stop=False)
                    mmv(ps[:], mod2[:, 768 + 128 * m:768 + 128 * m + 128],
                             sc2[1][:], start=False, stop=True)
                    V.tensor_scalar(mvec[:, m:m + 1], ps[:], tmodb2[:, m:m + 1],
                                    None, ALU.add)
                if debug and i == 0:
                    nc.sync.dma_start(d_dbg_cls[0:128, :].bitcast(F32R), cls[0][:])
                    nc.sync.dma_start(d_dbg_cls[128:256, :].bitcast(F32R), cls[1][:])
                    nc.sync.dma_start(d_dbg_mv[:, :], mvec[:, :])
                av = vp.tile([128, 2], F32, tag="av", name="av")
                bv = vp.tile([128, 2], F32R, tag="bv", name="bv")
                scr2 = vp.tile([128, 1], F32, tag="scr2", name="scr2")
                for d in range(2):
                    lcol = i * 6 + 2 * 2 + d
                    V.tensor_scalar(scr2[:], mvec[:, d:d + 1], 1.0, None, ALU.add)
                    V.tensor_tensor(av[:, d:d + 1], scr2[:],
                                    lngsb[:, lcol:lcol + 1], ALU.mult)
                    V.tensor_tensor(bv[:, d:d + 1], scr2[:],
                                    lnbsb[:, lcol:lcol + 1], ALU.mult)
                    V.tensor_tensor(bv[:, d:d + 1], bv[:, d:d + 1],
                                    mvec[:, 2 + d:3 + d], ALU.add)

                # bias_tot[f] = W1T.T @ bv + b1  (before scaling W1 in place)
                btot = vp.tile([128, 8], F32, tag="btot", name="btot")
                for m in range(8):
                    ps = pm.tile([128, 1], F32, tag="ps", name="ps")
                    mmv(ps[:], w1[:, 128 * m:128 * m + 128], bv[:, 0:1],
                             start=True, stop=False)
                    mmv(ps[:], w1[:, 1024 + 128 * m:1024 + 128 * m + 128],
                             bv[:, 1:2], start=False, stop=True)
                    V.tensor_scalar(btot[:, m:m + 1], ps[:], tb1[:, m:m + 1],
                                    None, ALU.add)
                # scale W1 in place by (1+s)*gamma
                for d in range(2):
                    V.tensor_scalar(w1[:, 1024 * d:1024 * d + 1024],
                                    w1[:, 1024 * d:1024 * d + 1024],
                                    av[:, d:d + 1], None, ALU.mult)

                # ---- x_hat chunks + token MLP + fused residual ----
                for (o, w) in CHUNKS:
                    mub = pm.tile([128, 512], F32, tag="ps", name="mub")
                    rb = pm.tile([128, 512], F32, tag="ps", name="rb")
                    T.matmul(mub[:, 0:w], onesrow[0:1, 0:128], mu_r[0:1, o:o + w],
                             start=True, stop=True)
                    T.matmul(rb[:, 0:w], onesrow[0:1, 0:128], r_r[0:1, o:o + w],
                             start=True, stop=True)
                    xh = []
                    for dt in range(2):
                        x_ = cp.tile([128, 512], F32R, tag=f"big{dt}", name=f"xh{dt}")
                        V.tensor_tensor(x_[:, 0:w], tokT[dt][:, o:o + w],
                                        mub[:, 0:w], ALU.subtract)
                        V.tensor_tensor(x_[:, 0:w], x_[:, 0:w], rb[:, 0:w],
                                        ALU.mult)
                        xh.append(x_)
                    A = apool.tile([128, 8 * 512], F32R, tag="A", name="A")
                    for m in range(8):
                        hp = ph1.tile([128, 512], F32, tag="h1", name="h1")
                        T.matmul(hp[:, 0:w], w1[:, 128 * m:128 * m + 128],
                                 xh[0][:, 0:w], start=True, stop=False)
                        T.matmul(hp[:, 0:w], w1[:, 1024 + 128 * m:1024 + 128 * m + 128],
                                 xh[1][:, 0:w], start=False, stop=True)
                        S.activation(A[:, 512 * m:512 * m + w], hp[:, 0:w],
                                     AF.Gelu, bias=btot[:, m:m + 1])
                    for d in range(2):
                        h2p = ph2.tile([128, 512], F32, tag="h2", name="h2")
                        for k in range(8):
                            T.matmul(h2p[:, 0:w],
                                     w2[:, 256 * k + 128 * d:256 * k + 128 * d + 128],
                                     A[:, 512 * k:512 * k + w],
                                     start=(k == 0), stop=False)
                        T.matmul(h2p[:, 0:w], b2row[0:1, 128 * d:128 * d + 128],
                                 onesrow[0:1, 0:w], start=False, stop=True)
                        gcol = 4 + d
                        V.scalar_tensor_tensor(tokT[d][:, o:o + w], h2p[:, 0:w],
                                               mvec[:, gcol:gcol + 1],
                                               tokT[d][:, o:o + w],
                                               ALU.mult, ALU.add)

            if debug:
                for dt in range(2):
                    nc.sync.dma_start(
                        d_dbg_tok[128 * dt:128 * dt + 128, :].bitcast(F32R),
                        tokT[dt][:, :])

            # ---------------- final LN + head ----------------
            ln_stats()
            fing = vp.tile([128, 2], F32, tag="fing", name="fing")
            nc.sync.dma_start(fing[:], col2(d_fing, 2))
            finb = vp.tile([128, 2], F32, tag="finb", name="finb")
            nc.sync.dma_start(finb[:], col2(d_finb, 2))
            outw = vp.tile([128, 4], F32R, tag="outw", name="outw")
            ld_split(outw[:], d_outwT, 2)
            outbs = vp.tile([C, 1], F32, tag="outbs", name="outbs")
            nc.sync.dma_start(outbs[:], d_outb[:, :])
            for (o, w) in CHUNKS:
                po = pm.tile([2, 512], F32, tag="ps", name="ps")
                mub = pm.tile([128, 512], F32, tag="ps", name="mub")
                rb = pm.tile([128, 512], F32, tag="ps", name="rb")
                T.matmul(mub[:, 0:w], onesrow[0:1, 0:128], mu_r[0:1, o:o + w],
                         start=True, stop=True)
                T.matmul(rb[:, 0:w], onesrow[0:1, 0:128], r_r[0:1, o:o + w],
                         start=True, stop=True)
                for dt in range(2):
                    x_ = cp.tile([128, 512], F32R, tag=f"big{dt}", name=f"xh{dt}")
                    V.tensor_tensor(x_[:, 0:w], tokT[dt][:, o:o + w],
                                    mub[:, 0:w], ALU.subtract)
                    V.tensor_tensor(x_[:, 0:w], x_[:, 0:w], rb[:, 0:w], ALU.mult)
                    V.tensor_scalar(x_[:, 0:w], x_[:, 0:w], fing[:, dt:dt + 1],
                                    finb[:, dt:dt + 1], ALU.mult, ALU.add)
                    T.matmul(po[:, 0:w], outw[:, 2 * dt:2 * dt + 2], x_[:, 0:w],
                             start=(dt == 0), stop=(dt == 1))
                ot = cp.tile([C, 512], F32, tag="osb", name="osb", bufs=1)
                V.tensor_scalar(ot[:, 0:w], po[:, 0:w], outbs[:, 0:1],
                                None, ALU.add)
                nc.sync.dma_start(d_outT[:, o:o + w], ot[:, 0:w])

    split_excess_waits(nc)
    return nc


_NC_CACHE = {}


def _get_nc(depth=DEPTH, debug=False):
    key = (depth, debug)
    if key not in _NC_CACHE:
        _NC_CACHE[key] = build_nc(depth, debug)
    return _NC_CACHE[key]


def _freqs_hilo():
    f32 = np.float32
    fr = np.exp(
        -np.log(10000.0) * np.arange(TE // 2, dtype=f32) / (TE // 2)
    ).astype(f32)
    hi = (fr.view(np.uint32) & np.uint32(0xFFFFF000)).view(f32)
    lo = (fr - hi).astype(f32)
    return np.stack([hi, lo], axis=1).astype(f32)


def _shared_inputs(inputs):
    f32 = np.float32
    sh = {
        "posT": np.ascontiguousarray(inputs["pos"][0].T.astype(f32)),
        "inwT": np.ascontiguousarray(inputs["in_w"].T.astype(f32)),
        "inb": inputs["in_b"].reshape(D, 1).astype(f32),
        "freqs": _freqs_hilo(),
        "tp1T": np.ascontiguousarray(inputs["tp1_w"].T.astype(f32)),
        "tp1b": inputs["tp1_b"].reshape(D, 1).astype(f32),
        "tp2T": np.ascontiguousarray(inputs["tp2_w"].T.astype(f32)),
        "tp2b": inputs["tp2_b"].reshape(D, 1).astype(f32),
        "clsv": inputs["cls_tok"].reshape(D, 1).astype(f32),
        "qkvoT": np.ascontiguousarray(
            np.stack(
                [
                    np.stack(
                        [
                            inputs["attn_in_w"][i][0:D].T,
                            inputs["attn_in_w"][i][D:2 * D].T,
                            inputs["attn_in_w"][i][2 * D:3 * D].T,
                            inputs["attn_out_w"][i].T,
                        ]
                    )
                    for i in range(DEPTH)
                ]
            ).astype(f32)
        ),
        "attnb": np.ascontiguousarray(
            np.stack(
                [
                    np.stack(
                        [
                            inputs["attn_in_b"][i][0:D],
                            inputs["attn_in_b"][i][D:2 * D],
                            inputs["attn_in_b"][i][2 * D:3 * D],
                            inputs["attn_out_b"][i],
                        ]
                    )
                    for i in range(DEPTH)
                ]
            ).astype(f32).reshape(DEPTH, 4, D, 1)
        ),
        "modT": np.ascontiguousarray(
            np.transpose(inputs["adaln_mod_w"], (0, 1, 3, 2)).astype(f32)
        ),
        "modb": inputs["adaln_mod_b"].astype(f32).reshape(DEPTH, 3, 3 * D, 1),
        "lng": inputs["adaln_ln_g"].astype(f32).reshape(DEPTH, 3, D, 1),
        "lnb": inputs["adaln_ln_b"].astype(f32).reshape(DEPTH, 3, D, 1),
        "w1T": np.ascontiguousarray(
            np.transpose(inputs["mlp_w1"], (0, 1, 3, 2)).astype(f32)
        ),
        "b1": inputs["mlp_b1"].astype(f32).reshape(DEPTH, 2, FF, 1),
        "w2T": np.ascontiguousarray(
            np.transpose(inputs["mlp_w2"], (0, 1, 3, 2)).astype(f32)
        ),
        "b2": inputs["mlp_b2"].astype(f32).reshape(DEPTH, 2, D, 1),
        "fing": inputs["fin_g"].reshape(D, 1).astype(f32),
        "finb": inputs["fin_b"].reshape(D, 1).astype(f32),
        "outwT": np.ascontiguousarray(inputs["out_w"].T.astype(f32)),
        "outb": inputs["out_b"].reshape(C, 1).astype(f32),
        "ident": np.eye(8, dtype=f32),
        "onessc": np.full((128, 1), 1.0 / 256, dtype=f32),
        "onesrow": np.ones((1, 512), dtype=f32),
        "qzero": np.zeros((128, 8), dtype=f32),
    }
    return sh


def kernel(**inputs):
    global LAST
    nc = _get_nc()
    sh = _shared_inputs(inputs)
    x_t = np.asarray(inputs["x_t"], dtype=np.float32)
    tv = np.asarray(inputs["t"]).astype(np.int32)
    in_maps = []
    for c in range(NCORES):
        m = dict(sh)
        m["xT"] = np.ascontiguousarray(x_t[c].T)
        m["tval"] = tv[c].reshape(1, 1)
        in_maps.append(m)
    res = run_bass_kernel_spmd(
        nc, in_maps, core_ids=list(range(NCORES)), trace=TRACE
    )
    LAST = res
    out = np.stack(
        [np.ascontiguousarray(res.results[c]["outT"].T) for c in range(NCORES)]
    ).astype(np.float32)
    return out

